# revision 1
# baseline (speedup 1.0000x reference)
"""CompresSAE topk-masking kernel for 8 Trainium2 NeuronCores.

Pipeline per core (data-parallel over batch, B_core rows):
  A) normalize x rows, transpose -> xnT split into bf16 hi/lo
  B) encoder e = xn @ We via 3-term split-bf16 matmul (fp32-grade);
     fused per-512-chunk screen: top-8 positive + top-8 negative values
     (+ chunk-local indices) per row -> 1024 candidates/row
  C) top-64-of-candidates per row via 8 rounds of (max8 + match_replace);
     masked candidate values = cand - zapped, signs restored
  D) decoder out = e_masked @ Wd: rebuild per-chunk dense e_masked rows by
     gpsimd local_scatter, PE-transpose to [E,B] tiles, bf16 matmul,
     accumulated in PSUM over E; PE-transpose the output back to [B,D].
"""
import sys

for p in ("/opt/trn_rl_repo", "/root/.axon_site/_ro/trn_rl_repo"):
    if p not in sys.path:
        sys.path.insert(0, p)

import numpy as np

from concourse import bass_utils, tile, bacc
import concourse.mybir as mybir
from concourse.masks import make_identity

dt = mybir.dt
P = 128
D = 768
KD = D // P          # 6 contraction tiles
CHUNK = 512          # E-chunk width (= screen subchunk)
NSWEEP = 2           # decoder B-half sweeps (PSUM capacity)
TOPK = 64


def build(B_core: int, E: int, dbg: bool = False):
    nblk = B_core // P
    nchunk = E // CHUNK
    bps = nblk // NSWEEP          # blocks per decoder sweep
    ncand = 16 * nchunk           # candidates per row

    nc = bacc.Bacc(trn_type="TRN2", target_bir_lowering=False, debug=False)

    d_x = nc.dram_tensor("x", [B_core, D], dt.float32, kind="ExternalInput").ap()
    d_We = nc.dram_tensor("We", [D, E], dt.float32, kind="ExternalInput").ap()
    d_Wd = nc.dram_tensor("Wd", [E, D], dt.float32, kind="ExternalInput").ap()
    d_out = nc.dram_tensor("out", [B_core, D], dt.float32, kind="ExternalOutput").ap()
    nchunk_ = E // CHUNK
    if dbg:
        d_cand = nc.dram_tensor("dbg_cand", [P, 16 * nchunk_], dt.float32, kind="ExternalOutput").ap()
        d_lidx = nc.dram_tensor("dbg_lidx", [P, 16 * nchunk_], dt.uint16, kind="ExternalOutput").ap()
        d_emc = nc.dram_tensor("dbg_emc", [P, 16 * nchunk_], dt.bfloat16, kind="ExternalOutput").ap()
        d_em0 = nc.dram_tensor("dbg_em0", [P, CHUNK], dt.bfloat16, kind="ExternalOutput").ap()

    with tile.TileContext(nc) as tc:
        with tc.tile_pool(name="consts", bufs=1) as consts, \
             tc.tile_pool(name="live", bufs=1) as live:
            ident_f = consts.tile([P, P], dt.float32)
            make_identity(nc, ident_f)
            ident_b = consts.tile([P, P], dt.bfloat16)
            make_identity(nc, ident_b)
            # sign pattern over candidate slots: +1 for pos-half (8), -1 neg
            signpat = consts.tile([P, ncand // 16, 16], dt.float32)
            nc.vector.memset(signpat[:, :, 0:8], 1.0)
            nc.vector.memset(signpat[:, :, 8:16], -1.0)

            # long-lived per-block arrays
            xh = [live.tile([P, KD, P], dt.bfloat16, tag=f"xh{b}", name=f"xh{b}") for b in range(nblk)]
            xl = [live.tile([P, KD, P], dt.bfloat16, tag=f"xl{b}", name=f"xl{b}") for b in range(nblk)]
            cand = [live.tile([P, ncand], dt.float32, tag=f"cand{b}", name=f"cand{b}") for b in range(nblk)]
            lidx = [live.tile([P, ncand], dt.uint16, tag=f"lidx{b}", name=f"lidx{b}") for b in range(nblk)]
            emcand = [live.tile([P, ncand], dt.bfloat16, tag=f"emc{b}", name=f"emc{b}") for b in range(nblk)]

            # ---------------- Phase A: normalize + transpose + split ------
            with tc.tile_pool(name="phA", bufs=2) as phA, \
                 tc.tile_pool(name="psA", bufs=4, space="PSUM") as psA:
                for b in range(nblk):
                    xb = phA.tile([P, D], dt.float32, tag="xb")
                    nc.gpsimd.dma_start(out=xb[:, :], in_=d_x[b * P:(b + 1) * P, :])
                    sq = phA.tile([P, D], dt.float32, tag="sq")
                    ss = phA.tile([P, 1], dt.float32, tag="ss")
                    nc.scalar.activation(sq[:, :], xb[:, :],
                                         mybir.ActivationFunctionType.Square,
                                         accum_out=ss[:, :])
                    nrm = phA.tile([P, 1], dt.float32, tag="nrm")
                    nc.scalar.activation(nrm[:, :], ss[:, :],
                                         mybir.ActivationFunctionType.Sqrt)
                    rn = phA.tile([P, 1], dt.float32, tag="rn")
                    nc.vector.reciprocal(rn[:, :], nrm[:, :])
                    xnb = phA.tile([P, D], dt.float32, tag="xnb")
                    nc.scalar.activation(xnb[:, :], xb[:, :],
                                         mybir.ActivationFunctionType.Copy,
                                         scale=rn[:, :])
                    # transpose 6 [128,128] tiles -> xh/xl (bf16 hi/lo)
                    for g in range(2):      # two psum packs of 3 tiles
                        pk = psA.tile([P, 3 * P], dt.float32, tag="psA")
                        for j in range(3):
                            k = g * 3 + j
                            nc.tensor.transpose(pk[:, j * P:(j + 1) * P],
                                                xnb[:, k * P:(k + 1) * P],
                                                ident_f[:, :])
                        for j in range(3):
                            k = g * 3 + j
                            nc.scalar.copy(out=xh[b][:, k, :],
                                           in_=pk[:, j * P:(j + 1) * P])
                            nc.vector.tensor_sub(out=xl[b][:, k, :],
                                                 in0=pk[:, j * P:(j + 1) * P],
                                                 in1=xh[b][:, k, :])

            # ---------------- Phase B: encoder + fused screen -------------
            with tc.tile_pool(name="wstage", bufs=2) as wstage, \
                 tc.tile_pool(name="whl", bufs=2) as whl, \
                 tc.tile_pool(name="scr", bufs=4) as scr, \
                 tc.tile_pool(name="psB", bufs=1, space="PSUM") as psB:
                pse = [psB.tile([P, CHUNK], dt.float32, tag=f"pse{b}", name=f"pse{b}")
                       for b in range(nblk)]
                for c in range(nchunk):
                    wf = wstage.tile([P, KD, CHUNK], dt.float32, tag="wf")
                    nc.gpsimd.dma_start(
                        out=wf[:, :, :],
                        in_=d_We[:, c * CHUNK:(c + 1) * CHUNK].rearrange(
                            "(k p) n -> p k n", p=P))
                    wh = whl.tile([P, KD, CHUNK], dt.bfloat16, tag="wh")
                    wl = whl.tile([P, KD, CHUNK], dt.bfloat16, tag="wl")
                    nc.vector.tensor_copy(out=wh[:, :, :], in_=wf[:, :, :])
                    nc.vector.tensor_sub(out=wl[:, :, :], in0=wf[:, :, :],
                                         in1=wh[:, :, :])
                    for b in range(nblk):
                        first = True
                        for k in range(KD):
                            for (sa, sw) in ((xh[b], wh), (xh[b], wl),
                                             (xl[b], wh)):
                                nc.tensor.matmul(
                                    pse[b][:, :], sa[:, k, :], sw[:, k, :],
                                    start=first,
                                    stop=(k == KD - 1 and sa is xl[b]))
                                first = False
                        # negated eviction for the negative-side screen
                        en = scr.tile([P, CHUNK], dt.float32, tag="en")
                        nc.scalar.activation(en[:, :], pse[b][:, :],
                                             mybir.ActivationFunctionType.Copy,
                                             scale=-1.0)
                        # screens: top-8 of e (pos) and of -e (neg)
                        nc.vector.max(out=cand[b][:, 16 * c:16 * c + 8],
                                      in_=pse[b][:, :])
                        nc.vector.max_index(out=lidx[b][:, 16 * c:16 * c + 8],
                                            in_max=cand[b][:, 16 * c:16 * c + 8],
                                            in_values=pse[b][:, :])
                        nc.vector.max(out=cand[b][:, 16 * c + 8:16 * c + 16],
                                      in_=en[:, :])
                        nc.vector.max_index(
                            out=lidx[b][:, 16 * c + 8:16 * c + 16],
                            in_max=cand[b][:, 16 * c + 8:16 * c + 16],
                            in_values=en[:, :])

            # ---------------- Phase C helper: top-64 of candidates --------
            def emit_phaseC(phC, b):
                s1 = phC.tile([P, ncand], dt.float32, tag="s1", name=f"s1_{b}")
                s2 = phC.tile([P, ncand], dt.float32, tag="s2", name=f"s2_{b}")
                cur = cand[b]
                dst = s1
                for r in range(TOPK // 8):
                    v8 = phC.tile([P, 8], dt.float32, tag="v8", name=f"v8_{b}_{r}")
                    nc.vector.max(out=v8[:, :], in_=cur[:, :])
                    nc.vector.match_replace(out=dst[:, :],
                                            in_to_replace=v8[:, :],
                                            in_values=cur[:, :],
                                            imm_value=0.0)
                    cur, dst = dst, (s2 if dst is s1 else s1)
                dd = phC.tile([P, ncand], dt.float32, tag="dd", name=f"dd_{b}")
                nc.vector.tensor_sub(out=dd[:, :], in0=cand[b][:, :],
                                     in1=cur[:, :])
                nc.vector.tensor_mul(
                    out=emcand[b][:, :], in0=dd[:, :],
                    in1=signpat[:, :, :].rearrange("p a b -> p (a b)"))
                if dbg and b == 0:
                    nc.gpsimd.dma_start(out=d_cand, in_=cand[0][:, :])
                    nc.gpsimd.dma_start(out=d_lidx, in_=lidx[0][:, :])
                    nc.gpsimd.dma_start(out=d_emc, in_=emcand[0][:, :])

            # ---------------- Phase D: decoder (with interleaved C) -------
            with tc.tile_pool(name="phC", bufs=2) as phC, \
                 tc.tile_pool(name="wdstage", bufs=2) as wdstage, \
                 tc.tile_pool(name="wdh", bufs=2) as wdhp, \
                 tc.tile_pool(name="emc", bufs=6) as emcp, \
                 tc.tile_pool(name="rhs", bufs=3) as rhsp, \
                 tc.tile_pool(name="tail", bufs=2) as tailp, \
                 tc.tile_pool(name="psD", bufs=1, space="PSUM") as psD, \
                 tc.tile_pool(name="psT", bufs=2, space="PSUM") as psT:
                EK = CHUNK // P   # 4 E-subtiles per chunk
                for sw in range(NSWEEP):
                    for bi in range(bps):
                        emit_phaseC(phC, sw * bps + bi)
                    pso = [psD.tile([P, bps * P], dt.float32, tag=f"pso{m}", name=f"pso{m}_{sw}")
                           for m in range(KD)]
                    for c in range(nchunk):
                        wdf = wdstage.tile([P, EK, D], dt.float32, tag="wdf")
                        nc.gpsimd.dma_start(
                            out=wdf[:, :, :],
                            in_=d_Wd[c * CHUNK:(c + 1) * CHUNK, :].rearrange(
                                "(k p) n -> p k n", p=P))
                        wdh = wdhp.tile([P, EK, D], dt.bfloat16, tag="wdh")
                        nc.vector.tensor_copy(out=wdh[:, :, :], in_=wdf[:, :, :])
                        # rebuild dense masked-e rows for this chunk + transpose
                        rhs = []
                        for es in range(EK):
                            pk = psT.tile([P, bps * P], dt.bfloat16, tag="psT", name=f"psT{sw}_{c}_{es}")
                            rhs.append((es, pk))
                        for bi in range(bps):
                            b = sw * bps + bi
                            em = emcp.tile([P, CHUNK], dt.bfloat16, tag="em")
                            nc.gpsimd.local_scatter(
                                em[:, :],
                                emcand[b][:, 16 * c:16 * c + 16],
                                lidx[b][:, 16 * c:16 * c + 16].bitcast(dt.int16),
                                channels=P, num_elems=CHUNK, num_idxs=16)
                            if dbg and b == 0 and c == 0:
                                nc.gpsimd.dma_start(out=d_em0, in_=em[:, :])
                            for (es, pk) in rhs:
                                nc.tensor.transpose(
                                    pk[:, bi * P:(bi + 1) * P],
                                    em[:, es * P:(es + 1) * P],
                                    ident_b[:, :])
                        rr = []
                        for (es, pk) in rhs:
                            rt = rhsp.tile([P, bps * P], dt.bfloat16,
                                           tag=f"rt{es}", name=f"rt{es}_{sw}_{c}")
                            nc.scalar.copy(out=rt[:, :], in_=pk[:, :])
                            rr.append(rt)
                        for m in range(KD):
                            for es in range(EK):
                                nc.tensor.matmul(
                                    pso[m][:, :],
                                    wdh[:, es, m * P:(m + 1) * P],
                                    rr[es][:, :],
                                    start=(c == 0 and es == 0),
                                    stop=(c == nchunk - 1 and es == EK - 1))
                    # tail: transpose out^T [D, bps*P] -> out rows
                    ot = [tailp.tile([P, bps * P], dt.float32, tag=f"ot{m}", name=f"ot{m}_{sw}")
                          for m in range(KD)]
                    for m in range(KD):
                        nc.scalar.copy(out=ot[m][:, :], in_=pso[m][:, :])
                    for bi in range(bps):
                        b = sw * bps + bi
                        ob = tailp.tile([P, D], dt.float32, tag="ob")
                        for g in range(2):
                            pk = psT.tile([P, 3 * P], dt.float32, tag="psT")
                            for j in range(3):
                                m = g * 3 + j
                                nc.tensor.transpose(
                                    pk[:, j * P:(j + 1) * P],
                                    ot[m][:, bi * P:(bi + 1) * P],
                                    ident_f[:, :])
                            nc.scalar.copy(out=ob[:, g * 3 * P:(g + 1) * 3 * P],
                                           in_=pk[:, :])
                        nc.gpsimd.dma_start(out=d_out[b * P:(b + 1) * P, :],
                                            in_=ob[:, :])

    nc.compile()
    return nc


_CACHE = {}


def _get(B_core, E):
    key = (B_core, E)
    if key not in _CACHE:
        _CACHE[key] = build(B_core, E)
    return _CACHE[key]


def kernel(x, encoder_w, encoder_b, decoder_w, k, n_cores=8):
    x = np.ascontiguousarray(np.asarray(x, dtype=np.float32))
    We = np.ascontiguousarray(np.asarray(encoder_w, dtype=np.float32))
    Wd = np.ascontiguousarray(np.asarray(decoder_w, dtype=np.float32))
    b = np.asarray(encoder_b)
    assert int(np.asarray(k)) == TOPK, f"kernel compiled for k={TOPK}"
    assert not np.any(b), "nonzero encoder_b not supported"
    B, Dd = x.shape
    E = We.shape[1]
    assert Dd == D and B % n_cores == 0
    B_core = B // n_cores

    nc = _get(B_core, E)
    in_maps = [{"x": x[i * B_core:(i + 1) * B_core], "We": We, "Wd": Wd}
               for i in range(n_cores)]
    res = bass_utils.run_bass_kernel_spmd(nc, in_maps,
                                          core_ids=list(range(n_cores)))
    return np.concatenate([res.results[i]["out"] for i in range(n_cores)],
                          axis=0)

